# revision 4
# baseline (speedup 1.0000x reference)
"""Trainium2 Bass kernel for MultiHeadLatentAttentionSimple.

Sharding: 8 cores = 4 batches x 2 head-groups (8 heads each).
Each core computes, for its (batch b, head-group g):
  Q^T = (Wq_g' q_b^T + bq_g')          [1024, 2048]   (Wq pre-scaled by 1/sqrt(dk))
  lat^T = (Wc^T k_b^T + bc)            [512, 2048]
  K^T_h = Wk_h^T lat^T                 [128, 2048]    (bk dropped: softmax-invariant)
  V = lat Wv_g                         [2048, 1024]   (bv folded into host constant)
  P^T = exp(K_h Q_h^T)                 (scores transposed; k on partitions)
  O^T_h = V_h^T P^T / den              den = softmax denominator per (h, q)
  Y_bg = O^T.T Wo_g                    [2048, 2048]   partial over head-group
Host: out[b] = Y_b0 + Y_b1 + (bv @ Wo + bo).
All matmuls fp32r (full-rate); accumulation fp32.
"""
import math
import os
import sys

sys.path.insert(0, "/opt/trn_rl_repo")

import numpy as np

import concourse.bacc as bacc
import concourse.bass_isa as bass_isa
import concourse.mybir as mybir
import concourse.tile as tile
from concourse.bass_utils import run_bass_kernel_spmd

try:
    import antenv.axon_hooks  # noqa: F401
except ImportError:
    # Environments whose antenv lacks the NTFF hook registry: inject a
    # stub so run_bass_kernel_spmd(trace=...) degrades to no-trace
    # instead of crashing on the import.
    import types

    import antenv

    _stub = types.ModuleType("antenv.axon_hooks")
    _stub._hook = None
    _stub.set_axon_ntff_profile_hook = lambda h: setattr(_stub, "_hook", h)
    _stub.get_axon_ntff_profile_hook = lambda: _stub._hook
    sys.modules["antenv.axon_hooks"] = _stub
    antenv.axon_hooks = _stub


def _register_ntff_hook():
    """Register the ctypes NTFF profile hook so trace=True works under axon."""
    import contextlib
    import ctypes

    import antenv.axon_hooks as ah

    if ah.get_axon_ntff_profile_hook() is not None:
        return
    so_path = "/opt/axon/libaxon_pjrt.so"
    if not os.path.exists(so_path):
        return
    lib = ctypes.CDLL(so_path)
    if not hasattr(lib, "axon_start_nrt_profile"):
        return
    lib.axon_start_nrt_profile.argtypes = [
        ctypes.POINTER(ctypes.c_int64),
        ctypes.c_size_t,
    ]
    lib.axon_start_nrt_profile.restype = ctypes.c_int64
    lib.axon_stop_nrt_profile.argtypes = [ctypes.c_char_p]
    lib.axon_stop_nrt_profile.restype = ctypes.c_int64

    @contextlib.contextmanager
    def _hook(output_dir, device_ids):
        import jax

        jax.devices()
        if device_ids:
            ids = (ctypes.c_int64 * len(device_ids))(*device_ids)
            rc = lib.axon_start_nrt_profile(ids, len(device_ids))
        else:
            rc = lib.axon_start_nrt_profile(None, 0)
        if rc != 0:
            raise RuntimeError(f"axon_start_nrt_profile rc={rc}")
        try:
            yield
        finally:
            n = lib.axon_stop_nrt_profile(str(output_dir).encode())
            print(f"ntff profile: {n} file(s) written to {output_dir}")

    ah.set_axon_ntff_profile_hook(_hook)

F32 = mybir.dt.float32
F32R = mybir.dt.float32r
Exp = mybir.ActivationFunctionType.Exp
Identity = mybir.ActivationFunctionType.Identity
Copy = mybir.ActivationFunctionType.Copy
MULT = mybir.AluOpType.mult
ADD = mybir.AluOpType.add

D_MODEL = 2048
NUM_HEADS = 16
D_K = 128
LATENT = 512
B, S = 4, 2048
N_CORES = 8
HG = NUM_HEADS // 2          # heads per group = 8
DG = HG * D_K                # d_model slice per group = 1024

LAST_RESULTS = None          # test.py reads exec_time_ns from here


def build():
    nc = bacc.Bacc(None, target_bir_lowering=False, debug=False)

    qT = nc.dram_tensor("qT", [D_MODEL, S], F32R, kind="ExternalInput")
    kT = nc.dram_tensor("kT", [D_MODEL, S], F32R, kind="ExternalInput")
    Wq = nc.dram_tensor("Wq", [D_MODEL, DG], F32R, kind="ExternalInput")
    Wc = nc.dram_tensor("Wc", [D_MODEL, LATENT], F32R, kind="ExternalInput")
    Wk = nc.dram_tensor("Wk", [LATENT, DG], F32R, kind="ExternalInput")
    Wv = nc.dram_tensor("Wv", [LATENT, DG], F32R, kind="ExternalInput")
    Wo = nc.dram_tensor("Wo", [DG, D_MODEL], F32R, kind="ExternalInput")
    bq_t = nc.dram_tensor("bq_t", [128, HG], F32, kind="ExternalInput")
    bc_t = nc.dram_tensor("bc_t", [128, LATENT // 128], F32, kind="ExternalInput")
    Y = nc.dram_tensor("Y", [S, D_MODEL], F32, kind="ExternalOutput")

    QT_sp = nc.dram_tensor("QT_sp", [DG, S], F32R, kind="Internal")
    OT_sp = nc.dram_tensor("OT_sp", [DG, S], F32R, kind="Internal")

    with tile.TileContext(nc) as tc:
        with (
            tc.tile_pool(name="const", bufs=1) as const,
            tc.tile_pool(name="latp", bufs=1) as latp,
        ):
            bq_sb = const.tile([128, HG], F32)
            bc_sb = const.tile([128, LATENT // 128], F32)
            nc.sync.dma_start(bq_sb[:], bq_t[:])
            nc.sync.dma_start(bc_sb[:], bc_t[:])
            # latent^T, resident across attention: [128, 4, 2048]
            lat_sb = latp.tile([128, LATENT // 128, S], F32R)

            # ---------------- Phase 1: Q^T projection (spill to DRAM) --------
            with (
                tc.tile_pool(name="wq", bufs=1) as wqp,
                tc.tile_pool(name="xchunk", bufs=2) as xch,
                tc.tile_pool(name="psq", bufs=8, space="PSUM") as psq,
                tc.tile_pool(name="qstage", bufs=3) as qst,
            ):
                wq_sb = wqp.tile([128, 16, DG], F32R)
                nc.sync.dma_start(
                    wq_sb[:], Wq[:].rearrange("(t p) m -> p t m", p=128)
                )
                for c in range(4):
                    q_c = xch.tile([128, 16, 512], F32R, tag="xc")
                    nc.sync.dma_start(
                        q_c[:],
                        qT[:, 512 * c:512 * (c + 1)].rearrange(
                            "(t p) n -> p t n", p=128
                        ),
                    )
                    for mt in range(HG):
                        ps = psq.tile([128, 512], F32)
                        for kt in range(16):
                            nc.tensor.matmul(
                                ps[:],
                                wq_sb[:, kt, 128 * mt:128 * (mt + 1)],
                                q_c[:, kt, :],
                                start=(kt == 0),
                                stop=(kt == 15),
                            )
                        st = qst.tile([128, 512], F32R)
                        nc.scalar.activation(
                            st[:], ps[:], Identity, bias=bq_sb[:, mt:mt + 1]
                        )
                        nc.sync.dma_start(
                            QT_sp[128 * mt:128 * (mt + 1), 512 * c:512 * (c + 1)],
                            st[:],
                        )

            # ---------------- Phase 2: latent^T projection (stays in SBUF) ---
            with (
                tc.tile_pool(name="wc", bufs=1) as wcp,
                tc.tile_pool(name="xchunk2", bufs=2) as xch2,
                tc.tile_pool(name="psl", bufs=8, space="PSUM") as psl,
            ):
                wc_sb = wcp.tile([128, 16, LATENT], F32R)
                nc.sync.dma_start(
                    wc_sb[:], Wc[:].rearrange("(t p) m -> p t m", p=128)
                )
                for c in range(4):
                    k_c = xch2.tile([128, 16, 512], F32R, tag="xc2")
                    nc.sync.dma_start(
                        k_c[:],
                        kT[:, 512 * c:512 * (c + 1)].rearrange(
                            "(t p) n -> p t n", p=128
                        ),
                    )
                    for mt in range(LATENT // 128):
                        ps = psl.tile([128, 512], F32)
                        for kt in range(16):
                            nc.tensor.matmul(
                                ps[:],
                                wc_sb[:, kt, 128 * mt:128 * (mt + 1)],
                                k_c[:, kt, :],
                                start=(kt == 0),
                                stop=(kt == 15),
                            )
                        nc.scalar.activation(
                            lat_sb[:, mt, 512 * c:512 * (c + 1)],
                            ps[:],
                            Identity,
                            bias=bc_sb[:, mt:mt + 1],
                        )

            # ---------------- Phases 3+4: attention ------------------------
            NLT = LATENT // 128  # 4 latent k-tiles
            with (
                tc.tile_pool(name="wk", bufs=1) as wkp,
                tc.tile_pool(name="wv", bufs=1) as wvp,
                tc.tile_pool(name="v4", bufs=1) as v4p,
                tc.tile_pool(name="kh", bufs=2) as khp,
                tc.tile_pool(name="qh", bufs=2) as qhp,
                tc.tile_pool(name="pp", bufs=4) as ppl,
                tc.tile_pool(name="den", bufs=2) as denp,
                tc.tile_pool(name="ost", bufs=2) as ostp,
                tc.tile_pool(name="pskv", bufs=2, space="PSUM") as pskv,
                tc.tile_pool(name="pss", bufs=2, space="PSUM") as pss,
                tc.tile_pool(name="pso", bufs=1, space="PSUM") as pso,
            ):
                wk_sb = wkp.tile([128, NLT, DG], F32R)
                wv_sb = wvp.tile([128, NLT, DG], F32R)
                nc.sync.dma_start(
                    wk_sb[:], Wk[:].rearrange("(t p) m -> p t m", p=128)
                )
                nc.sync.dma_start(
                    wv_sb[:], Wv[:].rearrange("(t p) m -> p t m", p=128)
                )
                for grp in range(2):
                    # V for 4 heads: [Sk(16x128), 512] native layout
                    v4_sb = v4p.tile([128, 16, 512], F32R, tag="v4")
                    for skt in range(16):
                        ps = pskv.tile([128, 512], F32, tag="kv")
                        for lt in range(NLT):
                            nc.tensor.matmul(
                                ps[:],
                                lat_sb[:, lt, 128 * skt:128 * (skt + 1)],
                                wv_sb[:, lt, 512 * grp:512 * (grp + 1)],
                                start=(lt == 0),
                                stop=(lt == NLT - 1),
                            )
                        nc.scalar.activation(v4_sb[:, skt, :], ps[:], Copy)
                    for hh in range(4):
                        h = 4 * grp + hh
                        # K^T for head h: [128, 2048]
                        kh_sb = khp.tile([128, S], F32R, tag="kh")
                        for ck in range(4):
                            ps = pskv.tile([128, 512], F32, tag="kv")
                            for lt in range(NLT):
                                nc.tensor.matmul(
                                    ps[:],
                                    wk_sb[:, lt, 128 * h:128 * (h + 1)],
                                    lat_sb[:, lt, 512 * ck:512 * (ck + 1)],
                                    start=(lt == 0),
                                    stop=(lt == NLT - 1),
                                )
                            nc.scalar.activation(
                                kh_sb[:, 512 * ck:512 * (ck + 1)], ps[:], Copy
                            )
                        qh_sb = qhp.tile([128, S], F32R, tag="qh")
                        nc.sync.dma_start(
                            qh_sb[:], QT_sp[128 * h:128 * (h + 1), :]
                        )
                        for c in range(2):  # q-chunks of 1024
                            ps_o = pso.tile([128, 1024], F32, tag="o")
                            den = denp.tile([128, 1024], F32, tag="den")
                            for kt in range(16):
                                ps_s = pss.tile([128, 1024], F32, tag="s")
                                for half in range(2):
                                    nc.tensor.matmul(
                                        ps_s[:, 512 * half:512 * (half + 1)],
                                        kh_sb[:, 128 * kt:128 * (kt + 1)],
                                        qh_sb[
                                            :,
                                            1024 * c + 512 * half:
                                            1024 * c + 512 * (half + 1),
                                        ],
                                        start=True,
                                        stop=True,
                                    )
                                p_t = ppl.tile([128, 1024], F32R, tag="p")
                                nc.scalar.activation(p_t[:], ps_s[:], Exp)
                                if kt == 1:
                                    nc.vector.tensor_tensor(
                                        den[:],
                                        prev_p[:].bitcast(F32),
                                        p_t[:].bitcast(F32),
                                        ADD,
                                    )
                                elif kt > 1:
                                    nc.vector.tensor_tensor(
                                        den[:], den[:], p_t[:].bitcast(F32), ADD
                                    )
                                prev_p = p_t
                                for half in range(2):
                                    nc.tensor.matmul(
                                        ps_o[:, 512 * half:512 * (half + 1)],
                                        v4_sb[:, kt, 128 * hh:128 * (hh + 1)],
                                        p_t[:, 512 * half:512 * (half + 1)],
                                        start=(kt == 0),
                                        stop=(kt == 15),
                                    )
                            den_bc = denp.tile([128, 1024], F32, tag="denbc")
                            nc.gpsimd.partition_all_reduce(
                                den_bc[:], den[:], 128, bass_isa.ReduceOp.add
                            )
                            recip = denp.tile([128, 1024], F32, tag="recip")
                            nc.vector.reciprocal(recip[:], den_bc[:])
                            o_st = ostp.tile([128, 1024], F32, tag="ost")
                            nc.vector.tensor_tensor(
                                o_st[:], ps_o[:], recip[:], MULT
                            )
                            nc.sync.dma_start(
                                OT_sp[
                                    128 * h:128 * (h + 1),
                                    1024 * c:1024 * (c + 1),
                                ],
                                o_st[:].bitcast(F32R),
                            )

        # ---------------- Phase 5: output projection ------------------------
        with (
            tc.tile_pool(name="wo", bufs=1) as wop,
            tc.tile_pool(name="otl", bufs=1) as otlp,
            tc.tile_pool(name="psy", bufs=8, space="PSUM") as psy,
            tc.tile_pool(name="yst", bufs=3) as ystp,
        ):
            wo_sb = wop.tile([128, HG, D_MODEL], F32R)
            nc.sync.dma_start(
                wo_sb[:], Wo[:].rearrange("(t p) m -> p t m", p=128)
            )
            ot_sb = otlp.tile([128, HG, S], F32R)
            nc.sync.dma_start(
                ot_sb[:], OT_sp[:].rearrange("(t p) q -> p t q", p=128)
            )
            for m in range(16):
                for nn in range(4):
                    ps = psy.tile([128, 512], F32)
                    for dt in range(HG):
                        nc.tensor.matmul(
                            ps[:],
                            ot_sb[:, dt, 128 * m:128 * (m + 1)],
                            wo_sb[:, dt, 512 * nn:512 * (nn + 1)],
                            start=(dt == 0),
                            stop=(dt == HG - 1),
                        )
                    yst = ystp.tile([128, 512], F32)
                    nc.vector.tensor_copy(yst[:], ps[:])
                    nc.sync.dma_start(
                        Y[128 * m:128 * (m + 1), 512 * nn:512 * (nn + 1)],
                        yst[:],
                    )

    nc.compile()
    return nc


def kernel(query, key, value, Wq, bq, Wc, bc, Wk, bk, Wv, bv, Wo, bo):
    global LAST_RESULTS
    query = np.ascontiguousarray(np.asarray(query, dtype=np.float32))
    key = np.ascontiguousarray(np.asarray(key, dtype=np.float32))
    Wq = np.asarray(Wq, np.float32)
    bq = np.asarray(bq, np.float32)
    Wc = np.ascontiguousarray(np.asarray(Wc, np.float32))
    bc = np.asarray(bc, np.float32)
    Wk = np.asarray(Wk, np.float32)
    Wv = np.asarray(Wv, np.float32)
    Wo = np.asarray(Wo, np.float32)
    bo = np.asarray(bo, np.float32)
    bv = np.asarray(bv, np.float32)

    scale = 1.0 / math.sqrt(D_K)
    Wq_s = (Wq.astype(np.float64) * scale).astype(np.float32)
    bq_s = (bq.astype(np.float64) * scale).astype(np.float32)

    bc_t = np.ascontiguousarray(bc.reshape(LATENT // 128, 128).T)

    in_maps = []
    for core in range(N_CORES):
        b, g = core // 2, core % 2
        sl = slice(DG * g, DG * (g + 1))
        in_maps.append(
            {
                "qT": np.ascontiguousarray(query[b].T),
                "kT": np.ascontiguousarray(key[b].T),
                "Wq": np.ascontiguousarray(Wq_s[:, sl]),
                "Wc": Wc,
                "Wk": np.ascontiguousarray(Wk[:, sl]),
                "Wv": np.ascontiguousarray(Wv[:, sl]),
                "Wo": np.ascontiguousarray(Wo[sl, :]),
                "bq_t": np.ascontiguousarray(bq_s[sl].reshape(HG, 128).T),
                "bc_t": bc_t,
            }
        )

    nc = build()
    trace = bool(os.environ.get("BASS_TRACE"))
    tmpdir = os.environ.get("BASS_TMPDIR") or None
    if trace:
        try:
            _register_ntff_hook()
        except Exception:
            pass
    res = run_bass_kernel_spmd(
        nc,
        in_maps,
        list(range(N_CORES)),
        trace=trace,
        tmpdir=tmpdir,
    )
    LAST_RESULTS = res

    cvec = (bv.astype(np.float64) @ Wo.astype(np.float64) + bo).astype(np.float32)
    out = np.empty((B, S, D_MODEL), np.float32)
    for b in range(B):
        out[b] = res.results[2 * b]["Y"] + res.results[2 * b + 1]["Y"] + cvec
    return out
